# revision 4
# baseline (speedup 1.0000x reference)
"""Trainium2 Bass kernel for nn_ChamferLossSelf (B=4, N=4096, D=3).

Math (per batch b):
  P[i,j] = ||g_i - p_j||^2   (cross);  P1 = ||g_i - g_j||^2, P2 = ||p_i - p_j||^2
  loss = sum_j min_i P + sum_i min_j P + sum_r (sort(minsP1) - sort(minsP2))^2
  where minsPk = per-point NN distance (diag excluded).

Sharding: batch b -> cores (2b, 2b+1).  Core 2b:  rows=gts, cross cols=preds,
self=gts.  Core 2b+1: rows=preds, cross cols=gts, self=preds.  Each core
computes its cross-matrix row-mins (summed -> partial) and its self-matrix
NN-distance vector (sorted on-device via a normalized-bitonic network).  An
8-core AllGather shares (sorted vector, partial, sum-of-squares); every core
then computes the 4 final scalars identically; the host reads core 0.

Distances via one K=24 bf16 matmul per tile: each f32 coordinate is split
exactly into 3 bf16 terms (h+m+l); product classes hh,hm,mh,hl,lh,mm plus
3-way bf16 splits of ||y||^2 (vs ones-lhs) and ||x||^2 (vs ones-rhs) give
~1e-5-accurate FULL distances (>= 0) in PSUM at 1 PE cycle/row.

PSUM row-min drain is split across engines per 128-row block (4 chunks of
[128,1024] fp32 PSUM):
  Z-blocks: ScalarE copies 2 chunks -> bf16 SBUF; VectorE folds the other 2
            with TT-min(psum, bf16) -> bf16, then one 2x bf16 reduce.
  Y-blocks: ScalarE copies all 4 chunks; VectorE does one 2x bf16 reduce.
Self-matrix diagonal is masked by GPSIMD affine_select on the bf16 copy
(ScalarE always drains the diagonal chunk).  bf16 rounding of the full
distances costs ~4e-3 relative on each min (tolerance is 2e-2).
"""

import numpy as np

import concourse.bass as bass
import concourse.bacc as bacc
import concourse.bass_isa as bass_isa
import concourse.tile as tile
from concourse import mybir
from concourse.bass_utils import run_bass_kernel_spmd

F32 = mybir.dt.float32
BF16 = mybir.dt.bfloat16
AX = mybir.AxisListType
OP = mybir.AluOpType
ACTF = mybir.ActivationFunctionType

N = 4096
NP, NT = 128, 32  # sort grid [partitions, free]; s = p*NT + t
N_CORES = 8
DIAG_BIG = 1.0e6
ALPHA = 1.0
Z_PER_JOB = 15  # of 32 row-blocks use the DVE-heavy Z flavor; rest Y (ACT-heavy)

# ---------------------------------------------------------------------------
# Sort network codegen: normalized bitonic (flip merges), all-ascending.
# Grid [128, 32], sort index s = p*32 + t.
# ---------------------------------------------------------------------------


def _plain_sel(axis_len, k):
    return [[2 * k, axis_len // (2 * k)], [1, k]]


def _sort_stages():
    ops = []
    layout = "G"

    def need(lay):
        nonlocal layout
        if layout != lay:
            ops.append(("transpose", "G2GT" if lay == "GT" else "GT2G"))
            layout = lay

    for m in range(1, 13):
        size = 1 << m
        if size <= NT:
            need("G")
            half = size // 2
            nblk = NT // size
            lo = ([[size, nblk], [1, half]], 0)
            hi = ([[size, nblk], [1, half]], half)
            lo_mir = ([[size, nblk], [-1, half]], size - 1)
            hi_mir = ([[size, nblk], [-1, half]], half - 1)
            ops.append(("stage", "G", [
                (lo, lo, lo_mir, "min", False),
                (hi, hi, hi_mir, "max", False),
            ]))
        else:
            need("GT")
            ops.append(("shuffle_rev",))
            sp = size // NT
            half = sp // 2
            nblk = NP // sp
            lo = ([[sp, nblk], [1, half]], 0)
            hi = ([[sp, nblk], [1, half]], half)
            lo_mir = ([[sp, nblk], [-1, half]], sp - 1)
            hi_mir = ([[sp, nblk], [-1, half]], half - 1)
            ops.append(("stage", "GT", [
                (lo, lo, lo_mir, "min", True),
                (hi, hi, hi_mir, "max", True),
            ]))
        k = size // 4
        while k >= 1:
            if k >= NT:
                need("GT")
                kp = k // NT
                sel = _plain_sel(NP, kp)
                ops.append(("stage", "GT", [
                    ((sel, 0), (sel, 0), (sel, kp), "min", False),
                    ((sel, kp), (sel, 0), (sel, kp), "max", False),
                ]))
            else:
                need("G")
                sel = _plain_sel(NT, k)
                ops.append(("stage", "G", [
                    ((sel, 0), (sel, 0), (sel, k), "min", False),
                    ((sel, k), (sel, 0), (sel, k), "max", False),
                ]))
            k //= 2
    need("G")
    return ops


def _sel_ap(t, sel, rowsz, nparts):
    pairs, off = sel
    return bass.AP(t.tensor, t.offset + off, [[rowsz, nparts]] + [list(p) for p in pairs])


def _emit_sort(nc, pool, psp, M, identf, sfx=""):
    """Sort the 4096 f32 values of grid M [128, 32] ascending (s = p*32+t).
    Returns the sorted G-layout grid tile."""
    G = [pool.tile([NP, NT], F32, tag="srt_g0", bufs=1, name="srt_g0"), pool.tile([NP, NT], F32, tag="srt_g1", bufs=1, name="srt_g1")]
    T = [pool.tile([NT, NP], F32, tag="srt_t0", bufs=1, name="srt_t0"), pool.tile([NT, NP], F32, tag="srt_t1", bufs=1, name="srt_t1")]
    R = pool.tile([NT, NP], F32, tag="srt_rev", bufs=1, name="srt_rev")
    nc.vector.tensor_copy(G[0][:], M[:])
    gi, ti = 0, 0
    lay = "G"
    for op in _sort_stages():
        if op[0] == "transpose":
            if op[1] == "G2GT":
                ps = psp.tile([NT, NP], F32, tag="tp", bufs=2)
                nc.tensor.transpose(ps[:], G[gi][:], identf[:])
                nc.scalar.copy(T[ti][:], ps[:])
                lay = "GT"
            else:
                ps = psp.tile([NP, NT], F32, tag="tp", bufs=2)
                nc.tensor.transpose(ps[:], T[ti][:], identf[0:NT, 0:NT])
                nc.scalar.copy(G[gi][:], ps[:])
                lay = "G"
        elif op[0] == "shuffle_rev":
            nc.vector.stream_shuffle(R[:], T[ti][:], mask=list(range(NT - 1, -1, -1)))
        else:
            _, slay, cxs = op
            assert slay == lay
            if lay == "G":
                cur, nxt = G[gi], G[1 - gi]
                rowsz, nparts = NT, NP
                gi = 1 - gi
            else:
                cur, nxt = T[ti], T[1 - ti]
                rowsz, nparts = NP, NT
                ti = 1 - ti
            for dst_sel, in0_sel, in1_sel, alu, in1_rev in cxs:
                src1 = R if in1_rev else cur
                nc.vector.tensor_tensor(
                    _sel_ap(nxt, dst_sel, rowsz, nparts),
                    _sel_ap(cur, in0_sel, rowsz, nparts),
                    _sel_ap(src1, in1_sel, rowsz, nparts),
                    op=OP.min if alu == "min" else OP.max,
                )
    assert lay == "G"
    return G[gi]


# ---------------------------------------------------------------------------
# Kernel program (SPMD: identical on all 8 cores; roles differ via inputs)
# ---------------------------------------------------------------------------

# K=24 class layout: (lhs block, rhs block) pairs, 3 rows each:
#  rows 0-2:  ones | yy h/m/l   rows 9-11: -2mA | hX   rows 18-20: -2mA | mX
#  rows 3-5:  -2hA | hX         rows 12-14:-2hA | lX   rows 21-23: xx h/m/l | ones
#  rows 6-8:  -2hA | mX         rows 15-17:-2lA | hX
K_ROWS = 24
LHS_ROWS = {"h": (3, 6, 12), "m": (9, 18), "l": (15,)}
RHS_ROWS = {"h": (3, 9, 15), "m": (6, 18), "l": (12,)}


def _emit_program(nc, repeats=1):
    a_pts = nc.dram_tensor("a_pts", [N, 3], F32, kind="ExternalInput")
    b_pts = nc.dram_tensor("b_pts", [N, 3], F32, kind="ExternalInput")
    out_t = nc.dram_tensor("out", [1, 4], F32, kind="ExternalOutput")

    zs = {t for t in range(NT)
          if ((t + 1) * Z_PER_JOB) // NT != (t * Z_PER_JOB) // NT}

    with tile.TileContext(nc) as tc:
        with (
            tc.tile_pool(name="const", bufs=1) as cst,
            tc.tile_pool(name="setup", bufs=1) as stp,
            tc.tile_pool(name="feat", bufs=1) as feat,
            tc.tile_pool(name="jobs", bufs=1) as jbs,
            tc.tile_pool(name="jpsum", bufs=1, space="PSUM") as jpsum,
            tc.tile_pool(name="tpsum", bufs=1, space="PSUM") as tpsum,
            tc.tile_pool(name="dram", bufs=1, space="DRAM") as dram,
        ):
          for _rep in range(repeats):
            sfx = f"_r{_rep}"
            # ---- constants
            identf = cst.tile([128, 128], F32)
            nc.vector.memset(identf[:], 0.0)
            nc.gpsimd.affine_select(
                identf[:], identf[:], pattern=[[-1, 128]],
                compare_op=OP.not_equal, fill=1.0, base=0, channel_multiplier=1,
            )
            identb = cst.tile([128, 128], BF16)
            nc.vector.memset(identb[:], 0.0)
            nc.gpsimd.affine_select(
                identb[:], identb[:], pattern=[[-1, 128]],
                compare_op=OP.not_equal, fill=1.0, base=0, channel_multiplier=1,
            )

            FL = feat.tile([K_ROWS, N], BF16)    # lhs features of A
            FRC = feat.tile([K_ROWS, N], BF16)   # rhs features of B (cross)
            FRS = feat.tile([K_ROWS, N], BF16)   # rhs features of A (self)
            nc.vector.memset(FL[0:3, :], 1.0)    # ones rows pair with yy splits
            ones3 = feat.tile([3, N], BF16, name=f"ones3{sfx}")
            nc.vector.memset(ones3[:], 1.0)      # ones rows pair with xx splits
            nc.sync.dma_start(FRC[21:24, :], ones3[:])
            nc.sync.dma_start(FRS[21:24, :], ones3[:])

            def put3(stage_bf, F, rows):
                """stage_bf [96,128] (partition d*32+b, free p) -> F[r:r+3, :]
                for each r in rows, col enum j = b*128+p (flat reshape DMA)."""
                for r in rows:
                    nc.sync.dma_start(F[r : r + 3, :], stage_bf[:])

            def setup_set(pts, tag, make_lhs, F_rhs):
                """Load a point set, build split features.  Returns xx grid
                [128, 32] f32 (xx[p, t] = |point enum t*128+p|^2)."""
                gb = stp.tile([128, 96], F32, name=f"gb_{tag}{sfx}")
                nc.sync.dma_start(gb[:], pts[:].rearrange("(p b) d -> p (b d)", p=128))
                # d-major copy: gd[p, d*32+b] = gb[p, b*3+d]
                gd = stp.tile([128, 96], F32, name=f"gd_{tag}{sfx}")
                nc.vector.tensor_copy(
                    gd[:].rearrange("p (d b) -> p d b", d=3),
                    bass.AP(gb.tensor, gb.offset, [[96, 128], [1, 3], [3, 32]]),
                )
                # norms (b-major): xx[p, b] = sum_d gb[p, 3b+d]^2
                sq = stp.tile([128, 96], F32, name=f"sq_{tag}{sfx}")
                nc.scalar.activation(sq[:], gb[:], ACTF.Square)
                xxg = stp.tile([128, 32], F32, name=f"xx_{tag}{sfx}")
                nc.vector.tensor_reduce(
                    xxg[:], sq[:].rearrange("p (b d) -> p b d", d=3),
                    axis=AX.X, op=OP.add,
                )
                # exact 3-way bf16 split of coordinates (d-major grids)
                h = stp.tile([128, 96], BF16, name=f"h_{tag}{sfx}")
                nc.vector.tensor_copy(h[:], gd[:])
                r1 = stp.tile([128, 96], F32, name=f"r1_{tag}{sfx}")
                nc.vector.tensor_tensor(r1[:], gd[:], h[:], op=OP.subtract)
                mg = stp.tile([128, 96], BF16, name=f"m_{tag}{sfx}")
                nc.vector.tensor_copy(mg[:], r1[:])
                r2 = stp.tile([128, 96], F32, name=f"r2_{tag}{sfx}")
                nc.vector.tensor_tensor(r2[:], r1[:], mg[:], op=OP.subtract)
                lg = stp.tile([128, 96], BF16, name=f"l_{tag}{sfx}")
                nc.vector.tensor_copy(lg[:], r2[:])

                splits = {"h": h, "m": mg, "l": lg}
                # transpose each split [128,96] -> [96,128] and DMA into F rows
                for s, grid in splits.items():
                    ps = tpsum.tile([96, 128], BF16, tag="tp", bufs=2)
                    nc.tensor.transpose(ps[:], grid[:], identb[:])
                    st = stp.tile([96, 128], BF16, name=f"st_{s}_{tag}{sfx}")
                    nc.scalar.copy(st[:], ps[:])
                    put3(st, F_rhs, RHS_ROWS[s])
                    if make_lhs:
                        st2 = stp.tile([96, 128], BF16, name=f"st2_{s}_{tag}{sfx}")
                        nc.vector.tensor_scalar(st2[:], st[:], -2.0, None, OP.mult)
                        put3(st2, FL, LHS_ROWS[s])
                # norm rows: transpose xx grid -> [32, 128], 3-way split.
                # rhs rows 0-2 get yy of this set; if make_lhs, FL rows 21-23
                # get xx of this set (same data).
                yps = tpsum.tile([32, 128], F32, tag="tp", bufs=2)
                nc.tensor.transpose(yps[:], xxg[:], identf[:])
                yst = stp.tile([32, 128], F32, name=f"yst_{tag}{sfx}")
                nc.scalar.copy(yst[:], yps[:])
                yh = stp.tile([32, 128], BF16, name=f"yh_{tag}{sfx}")
                nc.vector.tensor_copy(yh[:], yst[:])
                yr1 = stp.tile([32, 128], F32, name=f"yr1_{tag}{sfx}")
                nc.vector.tensor_tensor(yr1[:], yst[:], yh[:], op=OP.subtract)
                ym = stp.tile([32, 128], BF16, name=f"ym_{tag}{sfx}")
                nc.vector.tensor_copy(ym[:], yr1[:])
                yr2 = stp.tile([32, 128], F32, name=f"yr2_{tag}{sfx}")
                nc.vector.tensor_tensor(yr2[:], yr1[:], ym[:], op=OP.subtract)
                yl = stp.tile([32, 128], BF16, name=f"yl_{tag}{sfx}")
                nc.vector.tensor_copy(yl[:], yr2[:])
                for i, yt in enumerate((yh, ym, yl)):
                    nc.sync.dma_start(F_rhs[i : i + 1, :], yt[:])
                    if make_lhs:
                        nc.sync.dma_start(FL[21 + i : 22 + i, :], yt[:])
                return xxg

            xxA = setup_set(a_pts, "a", make_lhs=True, F_rhs=FRS)
            setup_set(b_pts, "b", make_lhs=False, F_rhs=FRC)

            # ---- distance jobs: rowmin over all 4096 cols per row.
            # PSUM holds full distances (xx included via rows 21-23), so the
            # drain can go through bf16 without precision loss near the min.
            def mm2(ps, lhsT, F_rhs, c):
                nc.tensor.matmul(
                    ps[:, 0:512], lhsT,
                    F_rhs[:, c * 1024 : c * 1024 + 512],
                    start=True, stop=True,
                )
                nc.tensor.matmul(
                    ps[:, 512:1024], lhsT,
                    F_rhs[:, c * 1024 + 512 : (c + 1) * 1024],
                    start=True, stop=True,
                )

            def diag_mask(bs):
                nc.gpsimd.affine_select(
                    bs, bs, pattern=[[-1, 128]],
                    compare_op=OP.not_equal, fill=DIAG_BIG,
                    base=0, channel_multiplier=1,
                )

            def job(F_rhs, diag, name):
                M = jbs.tile([128, 32], F32, name=f"M_{name}{sfx}")
                for t in range(32):
                    lhsT = FL[:, t * 128 : (t + 1) * 128]
                    dchunk = (t * 128) // 1024
                    doff = (t * 128) % 1024
                    if t in zs:
                        # Z: ACT drains 2 chunks (incl. diag), DVE TT-folds 2
                        act_chunks = [dchunk, (dchunk + 1) % 4] if diag else [0, 1]
                        dve_chunks = [c for c in range(4) if c not in act_chunks]
                        B = jbs.tile([128, 2048], BF16, tag="bz", bufs=2)
                        for slot, c in enumerate(act_chunks):
                            ps = jpsum.tile([128, 1024], F32, tag="jp", bufs=3)
                            mm2(ps, lhsT, F_rhs, c)
                            nc.scalar.copy(B[:, slot * 1024 : (slot + 1) * 1024], ps[:])
                        if diag:
                            diag_mask(B[:, doff : doff + 128])
                        m = jbs.tile([128, 2048], BF16, tag="mz", bufs=2)
                        for slot, c in enumerate(dve_chunks):
                            ps = jpsum.tile([128, 1024], F32, tag="jp", bufs=3)
                            mm2(ps, lhsT, F_rhs, c)
                            nc.vector.tensor_tensor(
                                m[:, slot * 1024 : (slot + 1) * 1024], ps[:],
                                B[:, slot * 1024 : (slot + 1) * 1024], op=OP.min,
                            )
                        nc.vector.tensor_reduce(
                            M[:, t : t + 1], m[:], axis=AX.X, op=OP.min
                        )
                    else:
                        # Y: ACT drains all 4 chunks, DVE one 2x bf16 reduce
                        B4 = jbs.tile([128, 4096], BF16, tag="by", bufs=2)
                        for c in range(4):
                            ps = jpsum.tile([128, 1024], F32, tag="jp", bufs=3)
                            mm2(ps, lhsT, F_rhs, c)
                            nc.scalar.copy(B4[:, c * 1024 : (c + 1) * 1024], ps[:])
                        if diag:
                            off = dchunk * 1024 + doff
                            diag_mask(B4[:, off : off + 128])
                        nc.vector.tensor_reduce(
                            M[:, t : t + 1], B4[:], axis=AX.X, op=OP.min
                        )
                return M

            Mself = job(FRS, True, "self")

            # ---- sum of squares of self mins
            msq = jbs.tile([128, 32], F32, name=f"msq{sfx}")
            nc.vector.tensor_tensor(msq[:], Mself[:], Mself[:], op=OP.mult)
            ssum = jbs.tile([128, 1], F32, name=f"ssum{sfx}")
            nc.vector.tensor_reduce(ssum[:], msq[:], axis=AX.X, op=OP.add)
            ssum_a = jbs.tile([128, 1], F32, name=f"ssum_a{sfx}")
            nc.gpsimd.partition_all_reduce(
                ssum_a[:], ssum[:], channels=128, reduce_op=bass_isa.ReduceOp.add
            )

            # ---- sort self mins (VectorE; overlaps the pool-lane cross job)
            SG = _emit_sort(nc, jbs, tpsum, Mself, identf, sfx)

            Mcross = job(FRC, False, "cross")

            # ---- partial scalar (sum of cross rowmins), all-partitions
            csum = jbs.tile([128, 1], F32, name=f"csum{sfx}")
            nc.vector.tensor_reduce(csum[:], Mcross[:], axis=AX.X, op=OP.add)
            csum_a = jbs.tile([128, 1], F32, name=f"csum_a{sfx}")
            nc.gpsimd.partition_all_reduce(
                csum_a[:], csum[:], channels=128, reduce_op=bass_isa.ReduceOp.add
            )

            # ---- payload: [sorted(4096), partial, ssum, pad...]
            pay = jbs.tile([1, 4104], F32, tag="pay", bufs=1, name="pay")
            nc.sync.dma_start(
                pay[0:1, 0:4096].rearrange("o (p t) -> o p t", p=128), SG[:]
            )
            nc.vector.tensor_copy(pay[0:1, 4096:4097], csum_a[0:1, :])
            nc.vector.tensor_copy(pay[0:1, 4097:4098], ssum_a[0:1, :])
            nc.vector.memset(pay[0:1, 4098:4104], 0.0)
            cc_in = dram.tile([1, 4104], F32)
            cc_out = dram.tile([N_CORES, 4104], F32, addr_space="Shared")
            nc.sync.dma_start(cc_in[:], pay[:])
            nc.gpsimd.collective_compute(
                "AllGather", OP.bypass,
                replica_groups=[list(range(N_CORES))],
                ins=[cc_in[:]], outs=[cc_out[:]],
            )

            # ---- final combine (identical on every core)
            sgs = []
            for c in range(N_CORES):
                g = jbs.tile([128, 32], F32, tag=f"fin_sg{c}", bufs=1, name=f"fin_sg{c}")
                nc.sync.dma_start(
                    g[:],
                    cc_out[c : c + 1, 0:4096].rearrange("o (p t) -> (o p) t", p=128),
                )
                sgs.append(g)
            scal = jbs.tile([8, 2], F32, name=f"fin_scal{sfx}")
            nc.sync.dma_start(scal[:], cc_out[:, 4096:4098])
            scrow = jbs.tile([1, 16], F32, name=f"fin_scrow{sfx}")
            nc.sync.dma_start(scrow[:], scal[:])
            drow = jbs.tile([1, 4], F32, name=f"fin_drow{sfx}")
            for b in range(4):
                pr = jbs.tile([128, 32], F32, tag="fin_pr", bufs=2)
                nc.vector.tensor_tensor(pr[:], sgs[2 * b][:], sgs[2 * b + 1][:], op=OP.mult)
                pc = jbs.tile([128, 1], F32, tag="fin_pc", bufs=2)
                nc.vector.tensor_reduce(pc[:], pr[:], axis=AX.X, op=OP.add)
                pa = jbs.tile([128, 1], F32, tag="fin_pa", bufs=2)
                nc.gpsimd.partition_all_reduce(
                    pa[:], pc[:], channels=128, reduce_op=bass_isa.ReduceOp.add
                )
                nc.vector.tensor_copy(drow[0:1, b : b + 1], pa[0:1, :])
            # out[b] = partial_2b + partial_2b+1 + ALPHA*(ss_2b + ss_2b+1 - 2*dot_b)
            t1 = jbs.tile([1, 4], F32, name=f"fin_t1{sfx}")
            nc.vector.tensor_tensor(
                t1[:],
                bass.AP(scrow.tensor, scrow.offset + 0, [[16, 1], [4, 4]]),
                bass.AP(scrow.tensor, scrow.offset + 2, [[16, 1], [4, 4]]),
                op=OP.add,
            )
            t2 = jbs.tile([1, 4], F32, name=f"fin_t2{sfx}")
            nc.vector.tensor_tensor(
                t2[:],
                bass.AP(scrow.tensor, scrow.offset + 1, [[16, 1], [4, 4]]),
                bass.AP(scrow.tensor, scrow.offset + 3, [[16, 1], [4, 4]]),
                op=OP.add,
            )
            t3 = jbs.tile([1, 4], F32, name=f"fin_t3{sfx}")
            # t3 = t1 + ALPHA * t2 ; ALPHA == 1.0
            nc.vector.tensor_tensor(t3[:], t1[:], t2[:], op=OP.add)
            res = jbs.tile([1, 4], F32, name=f"fin_res{sfx}")
            nc.vector.tensor_scalar(res[:], drow[:], -2.0 * ALPHA, None, OP.mult)
            nc.vector.tensor_tensor(res[:], res[:], t3[:], op=OP.add)
            nc.sync.dma_start(out_t[:], res[:])

    return nc


_CACHE = {}


def _get_nc(repeats=1):
    key = ("nc", repeats)
    if key not in _CACHE:
        nc = bacc.Bacc(
            "TRN2", target_bir_lowering=False, debug=False, num_devices=N_CORES
        )
        _emit_program(nc, repeats=repeats)
        nc.compile()
        _CACHE[key] = nc
    return _CACHE[key]


def make_in_maps(gts, preds):
    gts = np.ascontiguousarray(np.asarray(gts, dtype=np.float32))
    preds = np.ascontiguousarray(np.asarray(preds, dtype=np.float32))
    in_maps = []
    for c in range(N_CORES):
        b = c // 2
        if c % 2 == 0:
            a_set, b_set = gts[b], preds[b]
        else:
            a_set, b_set = preds[b], gts[b]
        in_maps.append(
            {"a_pts": np.ascontiguousarray(a_set), "b_pts": np.ascontiguousarray(b_set)}
        )
    return in_maps


def kernel(gts, preds):
    nc = _get_nc()
    in_maps = make_in_maps(gts, preds)
    res = run_bass_kernel_spmd(nc, in_maps, list(range(N_CORES)))
    return np.asarray(res.results[0]["out"][0], dtype=np.float32)
